# revision 1
# baseline (speedup 1.0000x reference)
"""Low-rank cross-attention on 8 Trainium2 NeuronCores (Bass/Tile).

Problem: out = (softmax((tgt@Wq.T)(memory@Wk.T).T / sqrt(r)) @ (memory@Wv.T)) @ Wo.T
Shapes: tgt/memory [4, 2048, 1024], r=128, d_model=1024.

Sharding: core c in 0..7 handles batch b=c//2 and query-half h=c%2
(1024 query tokens) against the full 2048-token memory of its batch.
No collectives.

Key layout trick: all DRAM inputs are host-pre-transposed so every
on-device matmul has its contraction dim on the SBUF partition axis:
  qT [r,T]   = WqT.T @ tgtT          (contract d)
  kT [r,S]   = WkT.T @ memT          (contract d)
  v  [S,e]   = memT.T @ WvT          (contract d)   <- natural lhsT use
  expT[S,Tq] = exp(scale * kT_s.T @ qT)             (contract r, single MM)
  UT [e,Tq]  = v_s.T @ expT          (contract S)
  out [T,o]  = UT_t.T @ WoT          (contract e)
Softmax: logits here are bounded (|x| < ~15), so exp is fp32-safe with
no max subtraction; row-sums come from a ones-vector matmul and the
division is folded into the final PSUM->SBUF scaling (per-partition
scalar multiply). All matmuls run in bf16 (inputs are cast host-side),
which lands the end-to-end error at the bf16 floor (~7.6e-3 rel).
"""

import ml_dtypes
import numpy as np

import concourse.bass as bass
import concourse.mybir as mybir
import concourse.tile as tile
from concourse.bacc import Bacc
from concourse.bass_utils import run_bass_kernel_spmd

FP = mybir.dt.float32
BF = mybir.dt.bfloat16
ts = bass.ts

B = 4
T_FULL = 2048
D = 1024
R = 128
S = 2048
E = 1024
O = 1024
T = 1024            # per-core query tokens (half of T_FULL)
P = 128
SCALE = 1.0 / np.sqrt(128.0)

KD = D // P         # 8 contraction tiles over d
NS = S // P         # 16 key/value tiles
NE = E // P         # 8 value-feature tiles
TQ = 256            # query-column strip processed per attention pass
NQ = T // TQ        # 4 strips

# Set by test harness to enable NTFF profiling; LAST_RESULT holds the
# BassKernelResults of the most recent kernel() call.
TRACE = False
LAST_RESULT = None
_PROG = None


def _build_program(linearize=False):
    # Bacc (not raw Bass): its finalize() runs move_matmul_waits_to_ldweights
    # + generate_event_semaphores, which split multi-sem waits down to the
    # one-wait-per-instruction limit of the TRN2 ISA. Raw Bass trips
    # walrus's "Too many sync wait commands" codegen error.
    nc = Bacc()

    tgtT_d = nc.dram_tensor("tgtT", [D, T], BF, kind="ExternalInput")
    memT_d = nc.dram_tensor("memT", [D, S], BF, kind="ExternalInput")
    wq_d = nc.dram_tensor("WqT", [D, R], BF, kind="ExternalInput")
    wk_d = nc.dram_tensor("WkT", [D, R], BF, kind="ExternalInput")
    wv_d = nc.dram_tensor("WvT", [D, E], BF, kind="ExternalInput")
    wo_d = nc.dram_tensor("WoT", [E, O], BF, kind="ExternalInput")
    out_d = nc.dram_tensor("out", [T, O], FP, kind="ExternalOutput")

    Exp = mybir.ActivationFunctionType.Exp

    with tile.TileContext(nc, linearize=linearize) as tc:
        with tc.tile_pool(name="perm", bufs=1) as perm, \
             tc.tile_pool(name="dram", bufs=1, space="DRAM") as dpool, \
             tc.tile_pool(name="expp", bufs=4) as expp, \
             tc.tile_pool(name="utsb", bufs=2) as utsb, \
             tc.tile_pool(name="outp", bufs=3) as outp, \
             tc.tile_pool(name="rcp", bufs=4) as rcp:
            qT = perm.tile([P, T], BF, tag="qT")
            kT = perm.tile([P, S], BF, tag="kT")
            v = [perm.tile([P, E], BF, tag=f"v{m}", name=f"v{m}") for m in range(NS)]
            ones = perm.tile([P, 1], BF, tag="ones")
            nc.vector.memset(ones, 1.0)
            recip_d = dpool.tile([1, T], FP)

            memT = [perm.tile([P, S], BF, tag=f"m{k}", name=f"m{k}") for k in range(KD)]
            wk = [perm.tile([P, R], BF, tag=f"wk{k}", name=f"wk{k}") for k in range(KD)]
            tgt = [perm.tile([P, T], BF, tag=f"t{k}", name=f"t{k}") for k in range(KD)]
            wq = [perm.tile([P, R], BF, tag=f"wq{k}", name=f"wq{k}") for k in range(KD)]
            wv = [perm.tile([P, E], BF, tag=f"wv{k}", name=f"wv{k}") for k in range(KD)]
            wo = [perm.tile([P, O], BF, tag=f"wo{k}", name=f"wo{k}") for k in range(NE)]
            # DMA bandwidth (~358 GB/s) is the Phase-A limiter at the start:
            # issue loads in exact consumption order (qT strips need wq +
            # tgt column strips; kT needs wk + memT strips; v needs wv; wo
            # is Phase-B only) so the PE ramps at ~3us instead of ~16us.
            for k in range(KD):
                nc.sync.dma_start(out=wq[k], in_=wq_d[ts(k, P), :])
            for k in range(KD):
                nc.sync.dma_start(out=tgt[k], in_=tgtT_d[ts(k, P), :])
            for k in range(KD):
                nc.sync.dma_start(out=wk[k], in_=wk_d[ts(k, P), :])
            for k in range(KD):
                nc.sync.dma_start(out=memT[k], in_=memT_d[ts(k, P), :])
            for k in range(KD):
                nc.sync.dma_start(out=wv[k], in_=wv_d[ts(k, P), :])
            for k in range(NE):
                nc.sync.dma_start(out=wo[k], in_=wo_d[ts(k, P), :])

            # ---- Phase A: projections (qT, kT, v) ----
            with tc.tile_pool(name="psA", bufs=4, space="PSUM") as psA:
                for n in range(T // 512):
                    ps = psA.tile([P, 512], FP)
                    for k in range(KD):
                        nc.tensor.matmul(ps, wq[k],
                                         tgt[k][:, ts(n, 512)],
                                         start=(k == 0), stop=(k == KD - 1))
                    nc.vector.tensor_copy(qT[:, ts(n, 512)], ps)

                for n in range(S // 512):
                    ps = psA.tile([P, 512], FP)
                    for k in range(KD):
                        nc.tensor.matmul(ps, wk[k],
                                         memT[k][:, ts(n, 512)],
                                         start=(k == 0), stop=(k == KD - 1))
                    nc.vector.tensor_copy(kT[:, ts(n, 512)], ps)

                for m in range(NS):
                    for eh in range(E // 512):
                        ps = psA.tile([P, 512], FP)
                        for k in range(KD):
                            nc.tensor.matmul(ps, memT[k][:, ts(m, P)],
                                             wv[k][:, ts(eh, 512)],
                                             start=(k == 0), stop=(k == KD - 1))
                        nc.vector.tensor_copy(v[m][:, ts(eh, 512)], ps)

            # ---- Phase B: attention + output projection, per 256-col strip ----
            with tc.tile_pool(name="psc", bufs=2, space="PSUM") as psc, \
                 tc.tile_pool(name="psums", bufs=1, space="PSUM") as psums, \
                 tc.tile_pool(name="psut", bufs=1, space="PSUM") as psut, \
                 tc.tile_pool(name="pso", bufs=1, space="PSUM") as pso:
                for q in range(NQ):
                    tq = slice(q * TQ, (q + 1) * TQ)
                    sums_ps = psums.tile([1, TQ], FP)
                    ut_ps = [psut.tile([P, 2 * TQ], FP, tag=f"ut{j}", name=f"ut{j}")
                             for j in range(NE // 2)]

                    def scores_exp(s, tq=tq):
                        sc = psc.tile([P, TQ], FP)
                        nc.tensor.matmul(sc, kT[:, ts(s, P)],
                                         qT[:, tq], start=True, stop=True)
                        ex = expp.tile([P, TQ], BF)
                        nc.scalar.activation(ex, sc, Exp, scale=float(SCALE))
                        return ex

                    ex_cur = scores_exp(0)
                    for s in range(NS):
                        ex_next = scores_exp(s + 1) if s + 1 < NS else None
                        first, last = (s == 0), (s == NS - 1)
                        nc.tensor.matmul(sums_ps, ones, ex_cur,
                                         start=first, stop=last)
                        for e in range(NE):
                            j, jj = divmod(e, 2)
                            # start=True clears has_written for the WHOLE
                            # PSUM bank; each ut bank holds two accumulation
                            # groups (jj=0,1), so only the first may clear.
                            # The jj=1 group's first matmul lands on cleared
                            # bits and overwrite+sets them (per-element
                            # accumulate semantics), which is exactly the
                            # start behavior it needs.
                            nc.tensor.matmul(ut_ps[j][:, ts(jj, TQ)],
                                             v[s][:, ts(e, P)], ex_cur,
                                             start=(first and jj == 0),
                                             stop=last)
                        ex_cur = ex_next

                    rcs = rcp.tile([1, TQ], FP, tag="rcs")
                    nc.vector.reciprocal(rcs, sums_ps)
                    nc.sync.dma_start(out=recip_d[0:1, tq], in_=rcs)

                    ut_sb = [utsb.tile([P, 2 * TQ], BF, tag=f"us{j}", name=f"us{j}")
                             for j in range(NE // 2)]
                    for j in range(NE // 2):
                        nc.vector.tensor_copy(ut_sb[j], ut_ps[j])

                    for tt in range(TQ // P):
                        tg = q * (TQ // P) + tt
                        rc = rcp.tile([P, 1], FP, tag="rc")
                        nc.sync.dma_start(
                            out=rc,
                            in_=recip_d[0:1, ts(tg, P)].rearrange("a b -> b a"))
                        for oh in range(O // 512):
                            po = pso.tile([P, 512], FP)
                            for e in range(NE):
                                j, jj = divmod(e, 2)
                                lhs = ut_sb[j][:, jj * TQ + tt * P:
                                               jj * TQ + (tt + 1) * P]
                                nc.tensor.matmul(po, lhs,
                                                 wo[e][:, ts(oh, 512)],
                                                 start=(e == 0), stop=(e == NE - 1))
                            ob = outp.tile([P, 512], FP)
                            nc.vector.tensor_scalar_mul(ob, po, rc)
                            nc.sync.dma_start(out=out_d[ts(tg, P), ts(oh, 512)],
                                              in_=ob)
    return nc


def kernel(tgt, memory, Wq, Wk, Wv, Wo):
    """8-way data-parallel (batch x query-half) low-rank cross-attention
    on the 8 NeuronCores via the Bass/Tile kernel above."""
    global LAST_RESULT, _PROG

    tgt = np.asarray(tgt, dtype=np.float32)
    memory = np.asarray(memory, dtype=np.float32)
    BFnp = ml_dtypes.bfloat16

    wqT = np.ascontiguousarray(np.asarray(Wq, np.float32).T).astype(BFnp)
    wkT = np.ascontiguousarray(np.asarray(Wk, np.float32).T).astype(BFnp)
    wvT = np.ascontiguousarray(np.asarray(Wv, np.float32).T).astype(BFnp)
    woT = np.ascontiguousarray(np.asarray(Wo, np.float32).T).astype(BFnp)

    in_maps = []
    for c in range(8):
        b, h = divmod(c, 2)
        tgtT = np.ascontiguousarray(
            tgt[b, h * T:(h + 1) * T, :].T).astype(BFnp)        # [D, T]
        memT = np.ascontiguousarray(memory[b].T).astype(BFnp)   # [D, S]
        in_maps.append({"tgtT": tgtT, "memT": memT,
                        "WqT": wqT, "WkT": wkT, "WvT": wvT, "WoT": woT})

    if _PROG is None:
        _PROG = _build_program()
        # Bacc defers register allocation to finalize(); the bass_exec
        # lowering serializes the module as-is, so finalize here or walrus
        # sees reg_id=-1 ("Reg has not been allocated yet").
        _PROG.finalize()
    res = run_bass_kernel_spmd(_PROG, in_maps, core_ids=list(range(8)),
                               trace=TRACE)
    LAST_RESULT = res

    out = np.empty((B, T_FULL, O), dtype=np.float32)
    for c in range(8):
        b, h = divmod(c, 2)
        out[b, h * T:(h + 1) * T, :] = res.results[c]["out"]
    return out



# revision 2
# speedup vs baseline: 1.1318x; 1.1318x over previous
"""Low-rank cross-attention on 8 Trainium2 NeuronCores (Bass/Tile).

Problem: out = (softmax((tgt@Wq.T)(memory@Wk.T).T / sqrt(r)) @ (memory@Wv.T)) @ Wo.T
Shapes: tgt/memory [4, 2048, 1024], r=128, d_model=1024.

Sharding: core c in 0..7 handles batch b=c//2 and query-half h=c%2
(1024 query tokens) against the full 2048-token memory of its batch.
No collectives.

Key layout trick: all DRAM inputs are host-pre-transposed so every
on-device matmul has its contraction dim on the SBUF partition axis:
  qT [r,T]   = WqT.T @ tgtT          (contract d)
  kT [r,S]   = WkT.T @ memT          (contract d)
  v  [S,e]   = memT.T @ WvT          (contract d)   <- natural lhsT use
  exT[S,Tq]  = exp(scale * kT_s.T @ qT)             (contract r, single MM)
  UT [e,Tq]  = v_s.T @ exT           (contract S)
  out [T,o]  = UT_t.T @ WoT          (contract e)

All Phase-B matmuls stream 512 columns so LDWEIGHTS fully hides under
the moving-data stream.  Softmax row-sums are kept OFF the PE: the
Vector engine accumulates exp tiles into a fp32 acc [128,512]; four
tiny fp32 matmuls (lhsT = acc 128-col block, rhs = ones) then yield the
TRANSPOSED per-query sums [128,1] directly in PSUM, so the reciprocal
needs no DRAM round-trip.  The division is folded into the final
PSUM->SBUF scaling.  Logits are bounded (|x| < ~15) so exp is fp32-safe
with no max subtraction.  All matmuls run in bf16 (inputs cast
host-side): end-to-end error ~7.6e-3 rel (bf16 floor).
"""

import ml_dtypes
import numpy as np

import concourse.bass as bass
import concourse.mybir as mybir
import concourse.tile as tile
from concourse.bacc import Bacc
from concourse.bass_utils import run_bass_kernel_spmd

FP = mybir.dt.float32
BF = mybir.dt.bfloat16
ts = bass.ts

B = 4
T_FULL = 2048
D = 1024
R = 128
S = 2048
E = 1024
O = 1024
T = 1024            # per-core query tokens (half of T_FULL)
P = 128
SCALE = 1.0 / np.sqrt(128.0)

KD = D // P         # 8 contraction tiles over d
NS = S // P         # 16 key/value tiles
NE = E // P         # 8 value-feature tiles
TQ = 512            # query-column strip processed per attention pass
NH = T // TQ        # 2 halves
NTT = TQ // P       # 4 query 128-blocks per half

# Set by test harness to enable NTFF profiling; LAST_RESULT holds the
# BassKernelResults of the most recent kernel() call.
TRACE = False
LAST_RESULT = None
_PROG = None


def _build_program(linearize=False):
    # Bacc (not raw Bass): its finalize() runs move_matmul_waits_to_ldweights
    # + generate_event_semaphores, which split multi-sem waits down to the
    # one-wait-per-instruction limit of the TRN2 ISA. Raw Bass trips
    # walrus's "Too many sync wait commands" codegen error.
    nc = Bacc()

    tgtT_d = nc.dram_tensor("tgtT", [D, T], BF, kind="ExternalInput")
    memT_d = nc.dram_tensor("memT", [D, S], BF, kind="ExternalInput")
    wq_d = nc.dram_tensor("WqT", [D, R], BF, kind="ExternalInput")
    wk_d = nc.dram_tensor("WkT", [D, R], BF, kind="ExternalInput")
    wv_d = nc.dram_tensor("WvT", [D, E], BF, kind="ExternalInput")
    wo_d = nc.dram_tensor("WoT", [E, O], BF, kind="ExternalInput")
    out_d = nc.dram_tensor("out", [T, O], FP, kind="ExternalOutput")

    Exp = mybir.ActivationFunctionType.Exp

    with tile.TileContext(nc, linearize=linearize) as tc:
        with tc.tile_pool(name="perm", bufs=1) as perm, \
             tc.tile_pool(name="expp", bufs=1) as expp, \
             tc.tile_pool(name="accp", bufs=1) as accp, \
             tc.tile_pool(name="utsb", bufs=1) as utsb, \
             tc.tile_pool(name="rcsb", bufs=1) as rcsb, \
             tc.tile_pool(name="outp", bufs=3) as outp:
            qT = perm.tile([P, T], BF, tag="qT")
            kT = perm.tile([P, S], BF, tag="kT")
            v = [perm.tile([P, E], BF, tag=f"v{m}", name=f"v{m}") for m in range(NS)]
            ones_f = perm.tile([P, 1], FP, tag="ones_f")
            nc.vector.memset(ones_f, 1.0)

            memT = [perm.tile([P, S], BF, tag=f"m{k}", name=f"m{k}") for k in range(KD)]
            wk = [perm.tile([P, R], BF, tag=f"wk{k}", name=f"wk{k}") for k in range(KD)]
            tgt = [perm.tile([P, T], BF, tag=f"t{k}", name=f"t{k}") for k in range(KD)]
            wq = [perm.tile([P, R], BF, tag=f"wq{k}", name=f"wq{k}") for k in range(KD)]
            wv = [perm.tile([P, E], BF, tag=f"wv{k}", name=f"wv{k}") for k in range(KD)]
            wo = [perm.tile([P, O], BF, tag=f"wo{k}", name=f"wo{k}") for k in range(NE)]

            # DMA bandwidth (~358 GB/s) is the Phase-A limiter at the start:
            # issue loads in exact consumption order.  PE order is
            # qT(half0) -> v(m ascending) -> qT(half1) -> kT -> attention, so:
            # wq, tgt half0, wv, memT m-block strips, tgt half1, wk, wo.
            for k in range(KD):
                nc.sync.dma_start(out=wq[k], in_=wq_d[ts(k, P), :])
            for k in range(KD):
                nc.sync.dma_start(out=tgt[k][:, 0:TQ], in_=tgtT_d[ts(k, P), 0:TQ])
            for k in range(KD):
                nc.sync.dma_start(out=wv[k], in_=wv_d[ts(k, P), :])
            # memT in m-pair column strips (v consumption order), k inner
            for mp in range(NS // 2):
                for k in range(KD):
                    nc.sync.dma_start(out=memT[k][:, ts(mp, 2 * P)],
                                      in_=memT_d[ts(k, P), ts(mp, 2 * P)])
                if mp == 1:
                    for k in range(KD):
                        nc.sync.dma_start(out=tgt[k][:, TQ:T],
                                          in_=tgtT_d[ts(k, P), TQ:T])
                if mp == 3:
                    for k in range(KD):
                        nc.sync.dma_start(out=wk[k], in_=wk_d[ts(k, P), :])
            for k in range(NE):
                nc.sync.dma_start(out=wo[k], in_=wo_d[ts(k, P), :])

            # ---- Phase A: projections (qT half0, v, qT half1, kT) ----
            with tc.tile_pool(name="psA", bufs=4, space="PSUM") as psA:
                def qT_group(n):
                    ps = psA.tile([P, TQ], FP)
                    for k in range(KD):
                        nc.tensor.matmul(ps, wq[k], tgt[k][:, ts(n, TQ)],
                                         start=(k == 0), stop=(k == KD - 1))
                    nc.vector.tensor_copy(qT[:, ts(n, TQ)], ps)

                qT_group(0)
                for m in range(NS):
                    for eh in range(E // TQ):
                        ps = psA.tile([P, TQ], FP)
                        for k in range(KD):
                            nc.tensor.matmul(ps, memT[k][:, ts(m, P)],
                                             wv[k][:, ts(eh, TQ)],
                                             start=(k == 0), stop=(k == KD - 1))
                        nc.vector.tensor_copy(v[m][:, ts(eh, TQ)], ps)
                    if m == 0:
                        qT_group(1)
                for n in range(S // TQ):
                    ps = psA.tile([P, TQ], FP)
                    for k in range(KD):
                        nc.tensor.matmul(ps, wk[k], memT[k][:, ts(n, TQ)],
                                         start=(k == 0), stop=(k == KD - 1))
                    nc.vector.tensor_copy(kT[:, ts(n, TQ)], ps)

            # ---- Phase B: attention + output projection, per 512-col half ----
            # PSUM budget (8 banks): psut 4 (ut accumulators, reused for the
            # second e-pass and again as out-proj accumulators) + psc 2
            # (score prefetch ping-pong) + rc 1.
            with tc.tile_pool(name="psc", bufs=2, space="PSUM") as psc, \
                 tc.tile_pool(name="psut", bufs=1, space="PSUM") as psut, \
                 tc.tile_pool(name="psrc", bufs=1, space="PSUM") as psrc:
                for h in range(NH):
                    tq = slice(h * TQ, (h + 1) * TQ)
                    ex = [expp.tile([P, TQ], BF, tag=f"ex{s}", name=f"ex{s}")
                          for s in range(NS)]
                    acc = accp.tile([P, TQ], FP, tag="acc")

                    def scores(s, tq=tq, ex=ex, acc=acc):
                        sc = psc.tile([P, TQ], FP)
                        nc.tensor.matmul(sc, kT[:, ts(s, P)], qT[:, tq],
                                         start=True, stop=True)
                        nc.scalar.activation(ex[s], sc, Exp, scale=float(SCALE))
                        if s == 0:
                            nc.vector.tensor_copy(acc, ex[s])
                        else:
                            nc.vector.tensor_add(acc, acc, ex[s])

                    # pass 1: e-blocks 0..3 accumulate over all s, with the
                    # scores/exp pipeline one s-tile ahead
                    ut1 = [psut.tile([P, TQ], FP, tag=f"ut{j}", name=f"ut{j}")
                           for j in range(4)]
                    scores(0)
                    for s in range(NS):
                        if s + 1 < NS:
                            scores(s + 1)
                        for e in range(4):
                            nc.tensor.matmul(ut1[e][:, :], v[s][:, ts(e, P)],
                                             ex[s], start=(s == 0),
                                             stop=(s == NS - 1))
                    ut_sb = [utsb.tile([P, TQ], BF, tag=f"us{e}", name=f"us{e}")
                             for e in range(NE)]
                    for e in range(4):
                        nc.vector.tensor_copy(ut_sb[e], ut1[e])

                    # pass 2: e-blocks 4..7 (all ex tiles now resident)
                    ut2 = [psut.tile([P, TQ], FP, tag=f"ut{j}", name=f"ut{j}2")
                           for j in range(4)]
                    for e in range(4):
                        for s in range(NS):
                            nc.tensor.matmul(ut2[e][:, :], v[s][:, ts(e + 4, P)],
                                             ex[s], start=(s == 0),
                                             stop=(s == NS - 1))
                        nc.vector.tensor_copy(ut_sb[e + 4], ut2[e])
                        if e == 0:
                            # transposed per-query sums: rc_ps[i, tt] =
                            # sum_p acc[p, tt*128+i]; the four single-column
                            # matmuls share one PSUM tile (start only on the
                            # first clears the bank's has_written bits).
                            rc_ps = psrc.tile([P, NTT], FP, tag="rc")
                            for tt in range(NTT):
                                nc.tensor.matmul(rc_ps[:, tt:tt + 1],
                                                 acc[:, ts(tt, P)], ones_f,
                                                 start=(tt == 0),
                                                 stop=(tt == NTT - 1))
                    rc = rcsb.tile([P, NTT], FP, tag="rc_sb")
                    nc.vector.reciprocal(rc, rc_ps)

                    # out projection: 8 groups of 8 accumulating matmuls,
                    # PSUM banks recycled from the ut pool
                    for g in range(NTT * (O // TQ)):
                        tt, oh = divmod(g, O // TQ)
                        po = psut.tile([P, TQ], FP, tag=f"ut{g % 2}",
                                       name=f"po{g}")
                        for e in range(NE):
                            nc.tensor.matmul(po, ut_sb[e][:, ts(tt, P)],
                                             wo[e][:, ts(oh, TQ)],
                                             start=(e == 0), stop=(e == NE - 1))
                        ob = outp.tile([P, TQ], FP)
                        nc.vector.tensor_scalar_mul(ob, po, rc[:, tt:tt + 1])
                        nc.sync.dma_start(
                            out=out_d[ts(h * NTT + tt, P), ts(oh, TQ)], in_=ob)
    return nc


def kernel(tgt, memory, Wq, Wk, Wv, Wo):
    """8-way data-parallel (batch x query-half) low-rank cross-attention
    on the 8 NeuronCores via the Bass/Tile kernel above."""
    global LAST_RESULT, _PROG

    tgt = np.asarray(tgt, dtype=np.float32)
    memory = np.asarray(memory, dtype=np.float32)
    BFnp = ml_dtypes.bfloat16

    wqT = np.ascontiguousarray(np.asarray(Wq, np.float32).T).astype(BFnp)
    wkT = np.ascontiguousarray(np.asarray(Wk, np.float32).T).astype(BFnp)
    wvT = np.ascontiguousarray(np.asarray(Wv, np.float32).T).astype(BFnp)
    woT = np.ascontiguousarray(np.asarray(Wo, np.float32).T).astype(BFnp)

    in_maps = []
    for c in range(8):
        b, h = divmod(c, 2)
        tgtT = np.ascontiguousarray(
            tgt[b, h * T:(h + 1) * T, :].T).astype(BFnp)        # [D, T]
        memT = np.ascontiguousarray(memory[b].T).astype(BFnp)   # [D, S]
        in_maps.append({"tgtT": tgtT, "memT": memT,
                        "WqT": wqT, "WkT": wkT, "WvT": wvT, "WoT": woT})

    if _PROG is None:
        _PROG = _build_program()
        # Bacc defers register allocation to finalize(); the bass_exec
        # lowering serializes the module as-is, so finalize here or walrus
        # sees reg_id=-1 ("Reg has not been allocated yet").
        _PROG.finalize()
    res = run_bass_kernel_spmd(_PROG, in_maps, core_ids=list(range(8)),
                               trace=TRACE)
    LAST_RESULT = res

    out = np.empty((B, T_FULL, O), dtype=np.float32)
    for c in range(8):
        b, h = divmod(c, 2)
        out[b, h * T:(h + 1) * T, :] = res.results[c]["out"]
    return out


# revision 3
# speedup vs baseline: 1.1649x; 1.0292x over previous
"""Low-rank cross-attention on 8 Trainium2 NeuronCores (Bass/Tile).

Problem: out = (softmax((tgt@Wq.T)(memory@Wk.T).T / sqrt(r)) @ (memory@Wv.T)) @ Wo.T
Shapes: tgt/memory [4, 2048, 1024], r=128, d_model=1024.

Sharding: core c in 0..7 handles batch b=c//2 and query-half h=c%2
(1024 query tokens) against the full 2048-token memory of its batch.
No collectives.

Key layout trick: all DRAM inputs are host-pre-transposed so every
on-device matmul has its contraction dim on the SBUF partition axis:
  qT [r,T]   = WqT.T @ tgtT          (contract d)
  kT [r,S]   = WkT.T @ memT          (contract d)
  v  [S,e]   = memT.T @ WvT          (contract d)   <- natural lhsT use
  exT[S,Tq]  = exp(scale * kT_s.T @ qT)             (contract r, single MM)
  UT [e,Tq]  = v_s.T @ exT           (contract S)
  out [T,o]  = UT_t.T @ WoT          (contract e)

All Phase-B matmuls stream 512 columns so LDWEIGHTS fully hides under
the moving-data stream.  Softmax row-sums are kept OFF the PE: the
Vector engine accumulates exp tiles into a fp32 acc [128,512]; four
tiny fp32 matmuls (lhsT = acc 128-col block, rhs = ones) then yield the
TRANSPOSED per-query sums [128,1] directly in PSUM, so the reciprocal
needs no DRAM round-trip.  The division is folded into the final
PSUM->SBUF scaling.  Logits are bounded (|x| < ~15) so exp is fp32-safe
with no max subtraction.  All matmuls run in bf16 (inputs cast
host-side): end-to-end error ~7.6e-3 rel (bf16 floor).
"""

import ml_dtypes
import numpy as np

import concourse.bass as bass
import concourse.mybir as mybir
import concourse.tile as tile
from concourse.bacc import Bacc
from concourse.bass_utils import run_bass_kernel_spmd

FP = mybir.dt.float32
BF = mybir.dt.bfloat16
ts = bass.ts

B = 4
T_FULL = 2048
D = 1024
R = 128
S = 2048
E = 1024
O = 1024
T = 1024            # per-core query tokens (half of T_FULL)
P = 128
SCALE = 1.0 / np.sqrt(128.0)

KD = D // P         # 8 contraction tiles over d
NS = S // P         # 16 key/value tiles
NE = E // P         # 8 value-feature tiles
TQ = 512            # query-column strip processed per attention pass
NH = T // TQ        # 2 halves
NTT = TQ // P       # 4 query 128-blocks per half

# Set by test harness to enable NTFF profiling; LAST_RESULT holds the
# BassKernelResults of the most recent kernel() call.
TRACE = False
LAST_RESULT = None
_PROG = None


def _build_program(linearize=False):
    # Bacc (not raw Bass): its finalize() runs move_matmul_waits_to_ldweights
    # + generate_event_semaphores, which split multi-sem waits down to the
    # one-wait-per-instruction limit of the TRN2 ISA. Raw Bass trips
    # walrus's "Too many sync wait commands" codegen error.
    nc = Bacc()

    tgtT_d = nc.dram_tensor("tgtT", [D, T], BF, kind="ExternalInput")
    memT_d = nc.dram_tensor("memT", [D, S], BF, kind="ExternalInput")
    wq_d = nc.dram_tensor("WqT", [D, R], BF, kind="ExternalInput")
    wk_d = nc.dram_tensor("WkT", [D, R], BF, kind="ExternalInput")
    wv_d = nc.dram_tensor("WvT", [D, E], BF, kind="ExternalInput")
    wo_d = nc.dram_tensor("WoT", [E, O], BF, kind="ExternalInput")
    out_d = nc.dram_tensor("out", [T, O], FP, kind="ExternalOutput")

    Exp = mybir.ActivationFunctionType.Exp

    with tile.TileContext(nc, linearize=linearize) as tc:
        with tc.tile_pool(name="perm", bufs=1) as perm, \
             tc.tile_pool(name="expp", bufs=1) as expp, \
             tc.tile_pool(name="accp", bufs=1) as accp, \
             tc.tile_pool(name="utsb", bufs=1) as utsb, \
             tc.tile_pool(name="rcsb", bufs=1) as rcsb, \
             tc.tile_pool(name="outp", bufs=3) as outp:
            qT = perm.tile([P, T], BF, tag="qT")
            kT = perm.tile([P, S], BF, tag="kT")
            v = [perm.tile([P, E], BF, tag=f"v{m}", name=f"v{m}") for m in range(NS)]
            ones_f = perm.tile([P, 1], FP, tag="ones_f")
            nc.vector.memset(ones_f, 1.0)

            memT = [perm.tile([P, S], BF, tag=f"m{k}", name=f"m{k}") for k in range(KD)]
            wk = [perm.tile([P, R], BF, tag=f"wk{k}", name=f"wk{k}") for k in range(KD)]
            tgt = [perm.tile([P, T], BF, tag=f"t{k}", name=f"t{k}") for k in range(KD)]
            wq = [perm.tile([P, R], BF, tag=f"wq{k}", name=f"wq{k}") for k in range(KD)]
            wv = [perm.tile([P, E], BF, tag=f"wv{k}", name=f"wv{k}") for k in range(KD)]
            wo = [perm.tile([P, O], BF, tag=f"wo{k}", name=f"wo{k}") for k in range(NE)]

            # Input DMA: only TWO hardware DGE queues exist (sync=SP and
            # scalar=Activation), each topping out well under HBM peak, so
            # split every load group across both queues (alternating k) and
            # keep per-partition rows >= 2KB ([128,1024]+ tiles) for packet
            # efficiency.  Issue order = consumption order: PE runs
            # qT(h0,h1) -> v(m ascending) -> kT, so: wq+tgt, wv, memT half0,
            # memT half1, wk, wo.
            def dma2(i, **kw):
                (nc.sync if i % 2 == 0 else nc.scalar).dma_start(**kw)

            for k in range(KD):
                dma2(k, out=wq[k], in_=wq_d[ts(k, P), :])
            for k in range(KD):
                dma2(k, out=tgt[k], in_=tgtT_d[ts(k, P), :])
            for k in range(KD):
                dma2(k, out=wv[k], in_=wv_d[ts(k, P), :])
            for sh in range(2):
                for k in range(KD):
                    dma2(k + sh, out=memT[k][:, ts(sh, S // 2)],
                         in_=memT_d[ts(k, P), ts(sh, S // 2)])
            for k in range(KD):
                dma2(k, out=wk[k], in_=wk_d[ts(k, P), :])
            for k in range(NE):
                dma2(k, out=wo[k], in_=wo_d[ts(k, P), :])

            # ---- Phase A: projections (qT, v, kT) ----
            with tc.tile_pool(name="psA", bufs=4, space="PSUM") as psA:
                for n in range(T // TQ):
                    ps = psA.tile([P, TQ], FP)
                    for k in range(KD):
                        nc.tensor.matmul(ps, wq[k], tgt[k][:, ts(n, TQ)],
                                         start=(k == 0), stop=(k == KD - 1))
                    nc.vector.tensor_copy(qT[:, ts(n, TQ)], ps)
                for m in range(NS):
                    for eh in range(E // TQ):
                        ps = psA.tile([P, TQ], FP)
                        for k in range(KD):
                            nc.tensor.matmul(ps, memT[k][:, ts(m, P)],
                                             wv[k][:, ts(eh, TQ)],
                                             start=(k == 0), stop=(k == KD - 1))
                        nc.vector.tensor_copy(v[m][:, ts(eh, TQ)], ps)
                for n in range(S // TQ):
                    ps = psA.tile([P, TQ], FP)
                    for k in range(KD):
                        nc.tensor.matmul(ps, wk[k], memT[k][:, ts(n, TQ)],
                                         start=(k == 0), stop=(k == KD - 1))
                    nc.vector.tensor_copy(kT[:, ts(n, TQ)], ps)

            # ---- Phase B: attention + output projection, per 512-col half ----
            # PSUM budget (8 banks): psut 4 (ut accumulators, reused for the
            # second e-pass and again as out-proj accumulators) + psc 2
            # (score prefetch ping-pong) + rc 1.
            with tc.tile_pool(name="psc", bufs=2, space="PSUM") as psc, \
                 tc.tile_pool(name="psut", bufs=1, space="PSUM") as psut, \
                 tc.tile_pool(name="psrc", bufs=1, space="PSUM") as psrc:
                for h in range(NH):
                    tq = slice(h * TQ, (h + 1) * TQ)
                    ex = [expp.tile([P, TQ], BF, tag=f"ex{s}", name=f"ex{s}")
                          for s in range(NS)]
                    acc = accp.tile([P, TQ], FP, tag="acc")

                    def scores(s, tq=tq, ex=ex, acc=acc):
                        sc = psc.tile([P, TQ], FP)
                        nc.tensor.matmul(sc, kT[:, ts(s, P)], qT[:, tq],
                                         start=True, stop=True)
                        nc.scalar.activation(ex[s], sc, Exp, scale=float(SCALE))
                        if s == 0:
                            nc.vector.tensor_copy(acc, ex[s])
                        else:
                            nc.vector.tensor_add(acc, acc, ex[s])

                    # pass 1: e-blocks 0..3 accumulate over all s, with the
                    # scores/exp pipeline one s-tile ahead
                    ut1 = [psut.tile([P, TQ], FP, tag=f"ut{j}", name=f"ut{j}")
                           for j in range(4)]
                    scores(0)
                    for s in range(NS):
                        if s + 1 < NS:
                            scores(s + 1)
                        for e in range(4):
                            nc.tensor.matmul(ut1[e][:, :], v[s][:, ts(e, P)],
                                             ex[s], start=(s == 0),
                                             stop=(s == NS - 1))
                    ut_sb = [utsb.tile([P, TQ], BF, tag=f"us{e}", name=f"us{e}")
                             for e in range(NE)]
                    for e in range(4):
                        nc.vector.tensor_copy(ut_sb[e], ut1[e])

                    # pass 2: e-blocks 4..7 (all ex tiles now resident)
                    ut2 = [psut.tile([P, TQ], FP, tag=f"ut{j}", name=f"ut{j}2")
                           for j in range(4)]
                    for e in range(4):
                        for s in range(NS):
                            nc.tensor.matmul(ut2[e][:, :], v[s][:, ts(e + 4, P)],
                                             ex[s], start=(s == 0),
                                             stop=(s == NS - 1))
                        nc.vector.tensor_copy(ut_sb[e + 4], ut2[e])
                        if e == 0:
                            # transposed per-query sums: rc_ps[i, tt] =
                            # sum_p acc[p, tt*128+i]; the four single-column
                            # matmuls share one PSUM tile (start only on the
                            # first clears the bank's has_written bits).
                            rc_ps = psrc.tile([P, NTT], FP, tag="rc")
                            for tt in range(NTT):
                                nc.tensor.matmul(rc_ps[:, tt:tt + 1],
                                                 acc[:, ts(tt, P)], ones_f,
                                                 start=(tt == 0),
                                                 stop=(tt == NTT - 1))
                    rc = rcsb.tile([P, NTT], FP, tag="rc_sb")
                    nc.vector.reciprocal(rc, rc_ps)

                    # out projection: 8 groups of 8 accumulating matmuls,
                    # PSUM banks recycled from the ut pool
                    for g in range(NTT * (O // TQ)):
                        tt, oh = divmod(g, O // TQ)
                        po = psut.tile([P, TQ], FP, tag=f"ut{g % 2}",
                                       name=f"po{g}")
                        for e in range(NE):
                            nc.tensor.matmul(po, ut_sb[e][:, ts(tt, P)],
                                             wo[e][:, ts(oh, TQ)],
                                             start=(e == 0), stop=(e == NE - 1))
                        ob = outp.tile([P, TQ], FP)
                        nc.vector.tensor_scalar_mul(ob, po, rc[:, tt:tt + 1])
                        nc.sync.dma_start(
                            out=out_d[ts(h * NTT + tt, P), ts(oh, TQ)], in_=ob)
    return nc


def kernel(tgt, memory, Wq, Wk, Wv, Wo):
    """8-way data-parallel (batch x query-half) low-rank cross-attention
    on the 8 NeuronCores via the Bass/Tile kernel above."""
    global LAST_RESULT, _PROG

    tgt = np.asarray(tgt, dtype=np.float32)
    memory = np.asarray(memory, dtype=np.float32)
    BFnp = ml_dtypes.bfloat16

    wqT = np.ascontiguousarray(np.asarray(Wq, np.float32).T).astype(BFnp)
    wkT = np.ascontiguousarray(np.asarray(Wk, np.float32).T).astype(BFnp)
    wvT = np.ascontiguousarray(np.asarray(Wv, np.float32).T).astype(BFnp)
    woT = np.ascontiguousarray(np.asarray(Wo, np.float32).T).astype(BFnp)

    in_maps = []
    for c in range(8):
        b, h = divmod(c, 2)
        tgtT = np.ascontiguousarray(
            tgt[b, h * T:(h + 1) * T, :].T).astype(BFnp)        # [D, T]
        memT = np.ascontiguousarray(memory[b].T).astype(BFnp)   # [D, S]
        in_maps.append({"tgtT": tgtT, "memT": memT,
                        "WqT": wqT, "WkT": wkT, "WvT": wvT, "WoT": woT})

    if _PROG is None:
        _PROG = _build_program()
        # Bacc defers register allocation to finalize(); the bass_exec
        # lowering serializes the module as-is, so finalize here or walrus
        # sees reg_id=-1 ("Reg has not been allocated yet").
        _PROG.finalize()
    res = run_bass_kernel_spmd(_PROG, in_maps, core_ids=list(range(8)),
                               trace=TRACE)
    LAST_RESULT = res

    out = np.empty((B, T_FULL, O), dtype=np.float32)
    for c in range(8):
        b, h = divmod(c, 2)
        out[b, h * T:(h + 1) * T, :] = res.results[c]["out"]
    return out


# revision 4
# speedup vs baseline: 1.1940x; 1.0250x over previous
"""Low-rank cross-attention on 8 Trainium2 NeuronCores (Bass/Tile).

Problem: out = (softmax((tgt@Wq.T)(memory@Wk.T).T / sqrt(r)) @ (memory@Wv.T)) @ Wo.T
Shapes: tgt/memory [4, 2048, 1024], r=128, d_model=1024.

Sharding: core c in 0..7 handles batch b=c//2 and query-half h=c%2
(1024 query tokens) against the full 2048-token memory of its batch.
No collectives.

Key layout trick: all DRAM inputs are host-pre-transposed so every
on-device matmul has its contraction dim on the SBUF partition axis:
  qT [r,T]   = WqT.T @ tgtT          (contract d)
  kT [r,S]   = WkT.T @ memT          (contract d)
  v  [S,e]   = memT.T @ WvT          (contract d)   <- natural lhsT use
  exT[S,Tq]  = exp(scale * kT_s.T @ qT)             (contract r, single MM)
  UT [e,Tq]  = v_s.T @ exT           (contract S)
  out [T,o]  = UT_t.T @ WoT          (contract e)

All Phase-B matmuls stream 512 columns so LDWEIGHTS fully hides under
the moving-data stream.  Softmax row-sums are kept OFF the PE: the
Vector engine accumulates exp tiles into a fp32 acc [128,512]; four
tiny fp32 matmuls (lhsT = acc 128-col block, rhs = ones) then yield the
TRANSPOSED per-query sums [128,1] directly in PSUM, so the reciprocal
needs no DRAM round-trip.  The division is folded into the final
PSUM->SBUF scaling.  Logits are bounded (|x| < ~15) so exp is fp32-safe
with no max subtraction.  All matmuls run in bf16 (inputs cast
host-side): end-to-end error ~7.6e-3 rel (bf16 floor).
"""

import ml_dtypes
import numpy as np

import concourse.bass as bass
import concourse.mybir as mybir
import concourse.tile as tile
from concourse.bacc import Bacc
from concourse.bass_utils import run_bass_kernel_spmd

FP = mybir.dt.float32
BF = mybir.dt.bfloat16
ts = bass.ts

B = 4
T_FULL = 2048
D = 1024
R = 128
S = 2048
E = 1024
O = 1024
T = 1024            # per-core query tokens (half of T_FULL)
P = 128
SCALE = 1.0 / np.sqrt(128.0)

KD = D // P         # 8 contraction tiles over d
NS = S // P         # 16 key/value tiles
NE = E // P         # 8 value-feature tiles
TQ = 512            # query-column strip processed per attention pass
NH = T // TQ        # 2 halves
NTT = TQ // P       # 4 query 128-blocks per half

# Set by test harness to enable NTFF profiling; LAST_RESULT holds the
# BassKernelResults of the most recent kernel() call.
TRACE = False
LAST_RESULT = None
_PROG = None


def _build_program(linearize=False):
    # Bacc (not raw Bass): its finalize() runs move_matmul_waits_to_ldweights
    # + generate_event_semaphores, which split multi-sem waits down to the
    # one-wait-per-instruction limit of the TRN2 ISA. Raw Bass trips
    # walrus's "Too many sync wait commands" codegen error.
    nc = Bacc()

    tgtT_d = nc.dram_tensor("tgtT", [D, T], BF, kind="ExternalInput")
    memT_d = nc.dram_tensor("memT", [D, S], BF, kind="ExternalInput")
    wq_d = nc.dram_tensor("WqT", [D, R], BF, kind="ExternalInput")
    wk_d = nc.dram_tensor("WkT", [D, R], BF, kind="ExternalInput")
    wv_d = nc.dram_tensor("WvT", [D, E], BF, kind="ExternalInput")
    wo_d = nc.dram_tensor("WoT", [E, O], BF, kind="ExternalInput")
    out_d = nc.dram_tensor("out", [T, O], FP, kind="ExternalOutput")

    Exp = mybir.ActivationFunctionType.Exp

    with tile.TileContext(nc, linearize=linearize) as tc:
        with tc.tile_pool(name="perm", bufs=1) as perm, \
             tc.tile_pool(name="expp", bufs=1) as expp, \
             tc.tile_pool(name="accp", bufs=1) as accp, \
             tc.tile_pool(name="utsb", bufs=1) as utsb, \
             tc.tile_pool(name="rcsb", bufs=1) as rcsb, \
             tc.tile_pool(name="outp", bufs=3) as outp:
            qT = perm.tile([P, T], BF, tag="qT")
            kT = perm.tile([P, S], BF, tag="kT")
            v = [perm.tile([P, E], BF, tag=f"v{m}", name=f"v{m}") for m in range(NS)]
            ones_f = perm.tile([P, 1], FP, tag="ones_f")
            nc.vector.memset(ones_f, 1.0)

            memT = [perm.tile([P, S], BF, tag=f"m{k}", name=f"m{k}") for k in range(KD)]
            wk = [perm.tile([P, R], BF, tag=f"wk{k}", name=f"wk{k}") for k in range(KD)]
            tgt = [perm.tile([P, T], BF, tag=f"t{k}", name=f"t{k}") for k in range(KD)]
            wq = [perm.tile([P, R], BF, tag=f"wq{k}", name=f"wq{k}") for k in range(KD)]
            wv = [perm.tile([P, E], BF, tag=f"wv{k}", name=f"wv{k}") for k in range(KD)]
            wo = [perm.tile([P, O], BF, tag=f"wo{k}", name=f"wo{k}") for k in range(NE)]

            # Input DMA: only TWO hardware DGE queues exist (sync=SP and
            # scalar=Activation), each topping out well under HBM peak, so
            # split every load group across both queues (alternating k) and
            # keep per-partition rows >= 2KB ([128,1024]+ tiles) for packet
            # efficiency.  Issue order = consumption order.  PE runs
            # kT (needs only wk + memT, 4.25MB, and is DMA-paced strip by
            # strip) -> v (wv has streamed in meanwhile) -> qT -> attention,
            # so: wk, memT half0, memT half1, wv, wq+tgt, wo.
            def dma2(i, **kw):
                (nc.sync if i % 2 == 0 else nc.scalar).dma_start(**kw)

            for k in range(KD):
                dma2(k, out=wk[k], in_=wk_d[ts(k, P), :])
            for sh in range(2):
                for k in range(KD):
                    dma2(k + sh, out=memT[k][:, ts(sh, S // 2)],
                         in_=memT_d[ts(k, P), ts(sh, S // 2)])
            for k in range(KD):
                dma2(k, out=wv[k], in_=wv_d[ts(k, P), :])
            for k in range(KD):
                dma2(k, out=wq[k], in_=wq_d[ts(k, P), :])
            for k in range(KD):
                dma2(k, out=tgt[k], in_=tgtT_d[ts(k, P), :])
            for k in range(NE):
                dma2(k, out=wo[k], in_=wo_d[ts(k, P), :])

            # ---- Phase A: projections (kT, v, qT) ----
            with tc.tile_pool(name="psA", bufs=4, space="PSUM") as psA:
                for n in range(S // TQ):
                    ps = psA.tile([P, TQ], FP)
                    for k in range(KD):
                        nc.tensor.matmul(ps, wk[k], memT[k][:, ts(n, TQ)],
                                         start=(k == 0), stop=(k == KD - 1))
                    nc.vector.tensor_copy(kT[:, ts(n, TQ)], ps)
                for m in range(NS):
                    for eh in range(E // TQ):
                        ps = psA.tile([P, TQ], FP)
                        for k in range(KD):
                            nc.tensor.matmul(ps, memT[k][:, ts(m, P)],
                                             wv[k][:, ts(eh, TQ)],
                                             start=(k == 0), stop=(k == KD - 1))
                        nc.vector.tensor_copy(v[m][:, ts(eh, TQ)], ps)
                for n in range(T // TQ):
                    ps = psA.tile([P, TQ], FP)
                    for k in range(KD):
                        nc.tensor.matmul(ps, wq[k], tgt[k][:, ts(n, TQ)],
                                         start=(k == 0), stop=(k == KD - 1))
                    nc.vector.tensor_copy(qT[:, ts(n, TQ)], ps)

            # ---- Phase B: attention + output projection, per 512-col half ----
            # PSUM budget (8 banks): psut 4 (ut accumulators, reused for the
            # second e-pass and again as out-proj accumulators) + psc 2
            # (score prefetch ping-pong) + rc 1.
            with tc.tile_pool(name="psc", bufs=2, space="PSUM") as psc, \
                 tc.tile_pool(name="psut", bufs=1, space="PSUM") as psut, \
                 tc.tile_pool(name="psrc", bufs=1, space="PSUM") as psrc:
                for h in range(NH):
                    tq = slice(h * TQ, (h + 1) * TQ)
                    ex = [expp.tile([P, TQ], BF, tag=f"ex{s}", name=f"ex{s}")
                          for s in range(NS)]
                    acc = accp.tile([P, TQ], FP, tag="acc")

                    def scores(s, tq=tq, ex=ex, acc=acc):
                        sc = psc.tile([P, TQ], FP)
                        nc.tensor.matmul(sc, kT[:, ts(s, P)], qT[:, tq],
                                         start=True, stop=True)
                        nc.scalar.activation(ex[s], sc, Exp, scale=float(SCALE))
                        if s == 0:
                            nc.vector.tensor_copy(acc, ex[s])
                        else:
                            nc.vector.tensor_add(acc, acc, ex[s])

                    # pass 1: e-blocks 0..3 accumulate over all s, with the
                    # scores/exp pipeline one s-tile ahead
                    ut1 = [psut.tile([P, TQ], FP, tag=f"ut{j}", name=f"ut{j}")
                           for j in range(4)]
                    scores(0)
                    for s in range(NS):
                        if s + 1 < NS:
                            scores(s + 1)
                        for e in range(4):
                            nc.tensor.matmul(ut1[e][:, :], v[s][:, ts(e, P)],
                                             ex[s], start=(s == 0),
                                             stop=(s == NS - 1))
                    ut_sb = [utsb.tile([P, TQ], BF, tag=f"us{e}", name=f"us{e}")
                             for e in range(NE)]
                    for e in range(4):
                        nc.vector.tensor_copy(ut_sb[e], ut1[e])

                    # pass 2: e-blocks 4..7 (all ex tiles now resident)
                    ut2 = [psut.tile([P, TQ], FP, tag=f"ut{j}", name=f"ut{j}2")
                           for j in range(4)]
                    for e in range(4):
                        for s in range(NS):
                            nc.tensor.matmul(ut2[e][:, :], v[s][:, ts(e + 4, P)],
                                             ex[s], start=(s == 0),
                                             stop=(s == NS - 1))
                        nc.vector.tensor_copy(ut_sb[e + 4], ut2[e])
                        if e == 0:
                            # transposed per-query sums: rc_ps[i, tt] =
                            # sum_p acc[p, tt*128+i]; the four single-column
                            # matmuls share one PSUM tile (start only on the
                            # first clears the bank's has_written bits).
                            rc_ps = psrc.tile([P, NTT], FP, tag="rc")
                            for tt in range(NTT):
                                nc.tensor.matmul(rc_ps[:, tt:tt + 1],
                                                 acc[:, ts(tt, P)], ones_f,
                                                 start=(tt == 0),
                                                 stop=(tt == NTT - 1))
                    rc = rcsb.tile([P, NTT], FP, tag="rc_sb")
                    nc.vector.reciprocal(rc, rc_ps)

                    # out projection: 8 groups of 8 accumulating matmuls,
                    # PSUM banks recycled from the ut pool
                    for g in range(NTT * (O // TQ)):
                        tt, oh = divmod(g, O // TQ)
                        po = psut.tile([P, TQ], FP, tag=f"ut{g % 2}",
                                       name=f"po{g}")
                        for e in range(NE):
                            nc.tensor.matmul(po, ut_sb[e][:, ts(tt, P)],
                                             wo[e][:, ts(oh, TQ)],
                                             start=(e == 0), stop=(e == NE - 1))
                        ob = outp.tile([P, TQ], FP)
                        nc.vector.tensor_scalar_mul(ob, po, rc[:, tt:tt + 1])
                        nc.sync.dma_start(
                            out=out_d[ts(h * NTT + tt, P), ts(oh, TQ)], in_=ob)
    return nc


def kernel(tgt, memory, Wq, Wk, Wv, Wo):
    """8-way data-parallel (batch x query-half) low-rank cross-attention
    on the 8 NeuronCores via the Bass/Tile kernel above."""
    global LAST_RESULT, _PROG

    tgt = np.asarray(tgt, dtype=np.float32)
    memory = np.asarray(memory, dtype=np.float32)
    BFnp = ml_dtypes.bfloat16

    wqT = np.ascontiguousarray(np.asarray(Wq, np.float32).T).astype(BFnp)
    wkT = np.ascontiguousarray(np.asarray(Wk, np.float32).T).astype(BFnp)
    wvT = np.ascontiguousarray(np.asarray(Wv, np.float32).T).astype(BFnp)
    woT = np.ascontiguousarray(np.asarray(Wo, np.float32).T).astype(BFnp)

    in_maps = []
    for c in range(8):
        b, h = divmod(c, 2)
        tgtT = np.ascontiguousarray(
            tgt[b, h * T:(h + 1) * T, :].T).astype(BFnp)        # [D, T]
        memT = np.ascontiguousarray(memory[b].T).astype(BFnp)   # [D, S]
        in_maps.append({"tgtT": tgtT, "memT": memT,
                        "WqT": wqT, "WkT": wkT, "WvT": wvT, "WoT": woT})

    if _PROG is None:
        _PROG = _build_program()
        # Bacc defers register allocation to finalize(); the bass_exec
        # lowering serializes the module as-is, so finalize here or walrus
        # sees reg_id=-1 ("Reg has not been allocated yet").
        _PROG.finalize()
    res = run_bass_kernel_spmd(_PROG, in_maps, core_ids=list(range(8)),
                               trace=TRACE)
    LAST_RESULT = res

    out = np.empty((B, T_FULL, O), dtype=np.float32)
    for c in range(8):
        b, h = divmod(c, 2)
        out[b, h * T:(h + 1) * T, :] = res.results[c]["out"]
    return out
